# revision 1
# baseline (speedup 1.0000x reference)
"""GANLoss kernel for Trainium2: out = -sum_i prob[i, target[i]] * reward[i].

Shapes: prob (8192, 32000) f32, target (8192,) int64, reward (8192,) f32.
Sharding: rows split across 8 NeuronCores (1024 rows/core).

Strategy: the loss touches only one element per row, so instead of streaming
the full 131 MB/core shard we dma_gather the 256-float (1 KB) chunk that
contains each row's target element (4 gather calls x 256 indices per core,
~1 MB read/core), then select the element with an iota/is_equal mask fused
with the reward multiply, and reduce on the vector engine. Each core emits
a [128, 4] tile of partial sums; the host sums them and negates.
"""

import numpy as np

N, C = 8192, 32000
N_CORES = 8
ROWS_PER_CORE = N // N_CORES          # 1024
N_GATHER = 4                          # gather calls per core
ROWS_PER_CALL = 256                   # idxs per gather call
ELEM = 256                            # f32 per gathered chunk (1 KB)
CHUNKS_PER_ROW = C // ELEM            # 125; max idx 255*125+124 = 31999 < 2^15

_cached = None


def _build_bass():
    import concourse.bacc as bacc
    import concourse.mybir as mybir

    f32 = mybir.dt.float32
    i16 = mybir.dt.int16

    nc = bacc.Bacc(num_swdge_queues=4)
    prob_d = nc.declare_dram_parameter("prob", [ROWS_PER_CORE, C], f32, isOutput=False)
    gidx_d = nc.declare_dram_parameter("gidx", [128, 16 * N_GATHER], i16, isOutput=False)
    offs_d = nc.declare_dram_parameter("offs", [128, 2 * N_GATHER], f32, isOutput=False)
    rew_d = nc.declare_dram_parameter("rew", [128, 2 * N_GATHER], f32, isOutput=False)
    out_d = nc.declare_dram_parameter("out", [128, N_GATHER], f32, isOutput=True)

    with (
        nc.sbuf_tensor([128, 16 * N_GATHER], i16) as idx_sb,
        nc.sbuf_tensor([128, 2 * N_GATHER], f32) as offs_sb,
        nc.sbuf_tensor([128, 2 * N_GATHER], f32) as rew_sb,
        nc.sbuf_tensor([128, ELEM], f32) as iota_sb,
        nc.sbuf_tensor([128, N_GATHER, 2, ELEM], f32) as gath_sb,
        nc.sbuf_tensor([128, 2 * ELEM], f32) as mask_sb,
        nc.sbuf_tensor([128, 2 * ELEM], f32) as prod_sb,
        nc.sbuf_tensor([128, N_GATHER], f32) as out_sb,
        nc.semaphore("in_sem") as in_sem,
        nc.semaphore("gs0") as gs0,
        nc.semaphore("gs1") as gs1,
        nc.semaphore("gs2") as gs2,
        nc.semaphore("gs3") as gs3,
        nc.semaphore("comp_sem") as comp_sem,
        nc.semaphore("ts_sem") as ts_sem,
        nc.semaphore("iota_sem") as iota_sem,
        nc.Block() as block,
    ):
        gsems = [gs0, gs1, gs2, gs3]

        @block.gpsimd
        def _(g):
            g.iota(
                iota_sb[:],
                pattern=[[1, ELEM]],
                base=0,
                channel_multiplier=0,
                allow_small_or_imprecise_dtypes=True,
            ).then_inc(iota_sem, 1)
            g.dma_start(idx_sb[:], gidx_d[:]).then_inc(in_sem, 16)
            g.dma_start(offs_sb[:], offs_d[:]).then_inc(in_sem, 16)
            g.dma_start(rew_sb[:], rew_d[:]).then_inc(in_sem, 16)
            g.wait_ge(in_sem, 48)
            for gi in range(N_GATHER):
                src = prob_d[ROWS_PER_CALL * gi : ROWS_PER_CALL * (gi + 1), :].rearrange(
                    "r (c e) -> (r c) e", e=ELEM
                )
                g.dma_gather(
                    gath_sb[:, gi],
                    src,
                    idx_sb[:, 16 * gi : 16 * (gi + 1)],
                    num_idxs=ROWS_PER_CALL,
                    num_idxs_reg=ROWS_PER_CALL,
                    elem_size=ELEM,
                    queue_num=gi,
                ).then_inc(gsems[gi], 16)
            g.wait_ge(comp_sem, N_GATHER)
            g.dma_start(out_d[:], out_sb[:]).then_inc(in_sem, 16)
            g.wait_ge(in_sem, 64)

        @block.vector
        def _(v):
            v.wait_ge(iota_sem, 1)
            v.wait_ge(in_sem, 48)
            for gi in range(N_GATHER):
                if gi > 0:
                    v.wait_ge(ts_sem, 3 * gi)  # prior mult done: mask free
                    v.wait_ge(comp_sem, gi)  # prior reduce done: prod free
                # maskrew[p, c*ELEM + w] = (w == t%ELEM) * reward  for row 256gi+128c+p
                v.tensor_scalar(
                    mask_sb[:, 0:ELEM],
                    iota_sb[:],
                    offs_sb[:, 2 * gi : 2 * gi + 1],
                    rew_sb[:, 2 * gi : 2 * gi + 1],
                    op0=mybir.AluOpType.is_equal,
                    op1=mybir.AluOpType.mult,
                ).then_inc(ts_sem, 1)
                v.tensor_scalar(
                    mask_sb[:, ELEM : 2 * ELEM],
                    iota_sb[:],
                    offs_sb[:, 2 * gi + 1 : 2 * gi + 2],
                    rew_sb[:, 2 * gi + 1 : 2 * gi + 2],
                    op0=mybir.AluOpType.is_equal,
                    op1=mybir.AluOpType.mult,
                ).then_inc(ts_sem, 1)
                v.wait_ge(ts_sem, 3 * gi + 2)
                v.wait_ge(gsems[gi], 16)
                v.tensor_mul(
                    prod_sb[:],
                    gath_sb[:, gi].rearrange("p a b -> p (a b)"),
                    mask_sb[:],
                ).then_inc(ts_sem, 1)
                v.wait_ge(ts_sem, 3 * gi + 3)
                v.tensor_reduce(
                    out_sb[:, gi : gi + 1],
                    prod_sb[:],
                    axis=mybir.AxisListType.X,
                    op=mybir.AluOpType.add,
                ).then_inc(comp_sem, 1)

    nc.compile()
    return nc


def _shard_host_inputs(prob, target, reward):
    """Per-core in_maps: prob shard + precomputed gather indices/offsets."""
    t_all = np.asarray(target).astype(np.int64)
    r_all = np.asarray(reward).astype(np.float32)
    prob = np.ascontiguousarray(np.asarray(prob, dtype=np.float32))
    in_maps = []
    loc = np.arange(ROWS_PER_CALL)
    for core in range(N_CORES):
        base = core * ROWS_PER_CORE
        t = t_all[base : base + ROWS_PER_CORE]
        r = r_all[base : base + ROWS_PER_CORE]
        chunk = (t // ELEM).astype(np.int64)
        off = (t % ELEM).astype(np.float32)
        gidx16 = np.zeros((16, 16 * N_GATHER), np.int16)
        offs = np.zeros((128, 2 * N_GATHER), np.float32)
        rew = np.zeros((128, 2 * N_GATHER), np.float32)
        for g in range(N_GATHER):
            rb = ROWS_PER_CALL * g
            idxv = loc * CHUNKS_PER_ROW + chunk[rb + loc]
            gidx16[loc % 16, 16 * g + loc // 16] = idxv.astype(np.int16)
            for ci in range(2):
                offs[:, 2 * g + ci] = off[rb + 128 * ci : rb + 128 * ci + 128]
                rew[:, 2 * g + ci] = r[rb + 128 * ci : rb + 128 * ci + 128]
        # the 8 GPSIMD cores each read their own 16-partition copy
        gidx = np.tile(gidx16, (8, 1))
        in_maps.append(
            {
                "prob": prob[base : base + ROWS_PER_CORE],
                "gidx": gidx,
                "offs": offs,
                "rew": rew,
            }
        )
    return in_maps


def kernel(prob, target, reward):
    global _cached
    from concourse.bass_utils import run_bass_kernel_spmd

    if _cached is None:
        _cached = _build_bass()
    nc = _cached
    in_maps = _shard_host_inputs(prob, target, reward)
    res = run_bass_kernel_spmd(nc, in_maps, list(range(N_CORES)))
    total = np.float64(0.0)
    for core_out in res.results:
        total += np.asarray(core_out["out"], dtype=np.float64).sum()
    return np.float32(-total)



# revision 29
# speedup vs baseline: 4.2499x; 4.2499x over previous
"""GANLoss kernel for Trainium2: out = -sum_i prob[i, target[i]] * reward[i].

Shapes: prob (8192, 32000) f32, target (8192,) int64, reward (8192,) f32.
Sharding: rows split across 8 NeuronCores (1024 rows/core).

Strategy: the loss touches one element per row. Per core, 8 dma_gather calls
of 128 idxs x 128 f32 (512 B) fetch the chunk holding each row's target
element; each call lands one row per partition, so one fused
scalar_tensor_tensor ((iota == offs) * chunk, accum_out=row-sum) per call
extracts the picked element for 128 rows in a single DVE pass. The small
input table (gather indices + offsets) is itself loaded with dma_gather via
an identity index table built on-device (two int32 bitwise ops — the only
integer ALU form the engines legally support), and the result leaves through
dma_scatter_add onto the runner's pre-zeroed output buffer — the kernel
issues no plain DMA at all, which avoids the slow HWDGE descriptor path and
its end-of-kernel drain penalty entirely. gpsimd runs all gathers + the
scatter; the vector engine runs the 8 selects. The device returns the 1024
picked probs per core; the host applies the reward weights and the final
negated sum.
"""

import numpy as np

N, C = 8192, 32000
N_CORES = 8
ROWS_PER_CORE = N // N_CORES          # 1024
N_CALLS = 8                           # prob gather calls per core
R = 128                               # idxs per gather call (1 row/partition)
ELEM = 128                            # f32 per gathered chunk (512 B)
CPR = C // ELEM                       # 250; max idx 127*250+249 = 31999 < 2^15

_cached = None


def _build_bass():
    import concourse.bacc as bacc
    import concourse.mybir as mybir
    from contextlib import ExitStack

    f32 = mybir.dt.float32
    i32 = mybir.dt.int32
    i16 = mybir.dt.int16
    A = mybir.AluOpType

    nc = bacc.Bacc(num_swdge_queues=4)
    prob_d = nc.declare_dram_parameter("prob", [ROWS_PER_CORE, C], f32, isOutput=False)
    # comb: cols 0:64 gather-idx table, cols 64:80 offsets (f32 bitcast), pad
    comb_d = nc.declare_dram_parameter("comb", [128, 128], i16, isOutput=False)
    out_d = nc.declare_dram_parameter("out", [128, 64], f32, isOutput=True)

    with ExitStack() as stack:
        e = stack.enter_context
        p1 = e(nc.sbuf_tensor([128, 4], i32))
        klo = e(nc.sbuf_tensor([128, 4], i32))
        s16 = e(nc.sbuf_tensor([128, 1], i32))
        m15 = e(nc.sbuf_tensor([128, 1], i32))
        t1 = e(nc.sbuf_tensor([128, 4], i32))
        idn32 = e(nc.sbuf_tensor([128, 4], i32))
        fio = e(nc.sbuf_tensor([128, ELEM], f32))
        comb_sb = e(nc.sbuf_tensor([128, 1, 128], i16))
        gath_sb = e(nc.sbuf_tensor([128, N_CALLS, ELEM], f32))
        junk_v = e(nc.sbuf_tensor([128, ELEM], f32))
        scat_sb = e(nc.sbuf_tensor([128, 1, 64], f32))
        ps = e(nc.semaphore("ps"))
        vq = e(nc.semaphore("vq"))
        idn = e(nc.semaphore("idn"))
        ld0 = e(nc.semaphore("ld0"))
        gs = [e(nc.semaphore(f"gs{i}")) for i in range(N_CALLS)]
        vs = e(nc.semaphore("vs"))
        fin = e(nc.semaphore("fin"))
        block = e(nc.Block())

        offs_ap = comb_sb[:, 0, 64:80].bitcast(f32)  # [128, 8]

        @block.gpsimd
        def _(g):
            # seeds for the identity idx table: the table must read
            # (p%16 + 16j) in every 16-partition block (each gpsimd core pair
            # consumes its own replica), packed as int32 pairs
            g.iota(p1[:], pattern=[[0, 4]], base=0, channel_multiplier=1).then_inc(
                ps, 1
            )
            g.iota(
                klo[:], pattern=[[32, 4]], base=1048576, channel_multiplier=0
            ).then_inc(ps, 1)
            g.iota(s16[:], pattern=[[0, 1]], base=16, channel_multiplier=0).then_inc(
                ps, 1
            )
            g.iota(m15[:], pattern=[[0, 1]], base=15, channel_multiplier=0).then_inc(
                ps, 1
            )
            g.iota(
                fio[:],
                pattern=[[1, ELEM]],
                base=0,
                channel_multiplier=0,
                allow_small_or_imprecise_dtypes=True,
            ).then_inc(ps, 1)
            g.memset(scat_sb[:, 0], 0.0).then_inc(ps, 1)
            g.wait_ge(idn, 1)
            g.dma_gather(
                comb_sb[:], comb_d[:], idn32[:].bitcast(i16),
                num_idxs=128, num_idxs_reg=128, elem_size=128, queue_num=0,
            ).then_inc(ld0, 16)
            g.wait_ge(ld0, 16)
            for i in range(N_CALLS):
                src = prob_d[R * i : R * (i + 1), :].rearrange(
                    "r (c e) -> (r c) e", e=ELEM
                )
                g.dma_gather(
                    gath_sb[:, i : i + 1], src,
                    comb_sb[:, 0, 8 * i : 8 * (i + 1)],
                    num_idxs=R, num_idxs_reg=R, elem_size=ELEM, queue_num=i % 4,
                ).then_inc(gs[i], 16)
            g.wait_ge(vs, N_CALLS)
            # output buffers are pre-zeroed by the runner on both the native
            # and PJRT paths, so the scatter-add lands the picked values as-is
            g.dma_scatter_add(
                out_d[:], scat_sb[:], idn32[:].bitcast(i16),
                num_idxs=128, num_idxs_reg=128, elem_size=64, queue_num=3,
            ).then_inc(fin, 16)

        @block.vector
        def _(v):
            # idn32[p, w] = ((p%16)+32w+16)<<16 | ((p%16)+32w): as int16 pairs
            # this is exactly idn[p, j] = p%16 + 16j
            v.wait_ge(ps, 4)
            v.scalar_tensor_tensor(
                t1[:], p1[:], m15[:], klo[:],
                op0=A.bitwise_and, op1=A.bitwise_or,
            ).then_inc(vq, 1)
            v.wait_ge(vq, 1)
            v.scalar_tensor_tensor(
                idn32[:], t1[:], s16[:], t1[:],
                op0=A.arith_shift_left, op1=A.bitwise_or,
            ).then_inc(idn, 1)
            v.wait_ge(ld0, 16)
            v.wait_ge(ps, 6)
            for i in range(N_CALLS):
                v.wait_ge(gs[i], 16)
                if i > 0:
                    v.wait_ge(vs, i)
                v.scalar_tensor_tensor(
                    junk_v[:],
                    fio[:],
                    offs_ap[:, i : i + 1],
                    gath_sb[:, i],
                    op0=A.is_equal,
                    op1=A.mult,
                    accum_out=scat_sb[:, 0, i : i + 1],
                ).then_inc(vs, 1)

    nc.compile()
    return nc


def _shard_host_inputs(prob, target, reward):
    """Per-core in_maps: prob shard + combined gather-idx/offset table."""
    t_all = np.asarray(target).astype(np.int64)
    prob = np.ascontiguousarray(np.asarray(prob, dtype=np.float32))
    loc = np.arange(R)
    in_maps = []
    for core in range(N_CORES):
        base = core * ROWS_PER_CORE
        t = t_all[base : base + ROWS_PER_CORE]
        chunk = (t // ELEM).astype(np.int64)
        off = (t % ELEM).astype(np.float32)
        gidx16 = np.zeros((16, 64), np.int16)
        offs = np.zeros((128, N_CALLS), np.float32)
        for i in range(N_CALLS):
            rb = R * i
            idxv = loc * CPR + chunk[rb + loc]
            gidx16[loc % 16, 8 * i + loc // 16] = idxv.astype(np.int16)
            offs[:, i] = off[rb : rb + 128]
        comb = np.zeros((128, 128), np.int16)
        comb[:, 0:64] = np.tile(gidx16, (8, 1))
        comb[:, 64:80] = offs.view(np.int16)
        in_maps.append(
            {
                "prob": prob[base : base + ROWS_PER_CORE],
                "comb": comb,
            }
        )
    return in_maps


def kernel(prob, target, reward):
    global _cached
    from concourse.bass_utils import run_bass_kernel_spmd

    if _cached is None:
        _cached = _build_bass()
    nc = _cached
    in_maps = _shard_host_inputs(prob, target, reward)
    res = run_bass_kernel_spmd(nc, in_maps, list(range(N_CORES)))
    r_all = np.asarray(reward, dtype=np.float64)
    total = np.float64(0.0)
    for core, core_out in enumerate(res.results):
        picked = np.asarray(core_out["out"], dtype=np.float64)[:, :N_CALLS]
        rew = r_all[core * ROWS_PER_CORE : (core + 1) * ROWS_PER_CORE]
        # picked[p, i] belongs to row 128*i + p of this core's shard
        total += (picked * rew.reshape(N_CALLS, 128).T).sum()
    return np.float32(-total)


# revision 35
# speedup vs baseline: 5.0510x; 1.1885x over previous
"""GANLoss kernel for Trainium2: out = -sum_i prob[i, target[i]] * reward[i].

Shapes: prob (8192, 32000) f32, target (8192,) int64, reward (8192,) f32.
Sharding: rows split across 8 NeuronCores (1024 rows/core).

Strategy: the loss touches one element per row. Per core, 8 dma_gather calls
of 128 idxs x 128 f32 (512 B) fetch the chunk holding each row's target
element; each call lands one row per partition, so one fused
scalar_tensor_tensor ((iota == offs) * chunk, accum_out=row-sum) on the
vector engine extracts the picked element for 128 rows in one pass. The
small input table (gather indices + offsets) is itself loaded with
dma_gather via an identity index table built on gpsimd in f32 (iota +
is_ge ladder for p%16, then a converting copy to int16 — the only integer
path the engines legally support), and the result leaves through
dma_scatter_add onto the runner's pre-zeroed output buffer — the kernel
issues no plain DMA at all, dodging the HWDGE descriptor path and its
end-of-kernel drain penalty. gpsimd runs the gathers, the last two selects'
mask/product passes, and the scatter; the vector engine runs six fused
selects plus two fast accumulate passes over gpsimd's products. The device
returns the 1024 picked probs per core; the host applies the reward weights
and the final negated sum.
"""

import numpy as np

N, C = 8192, 32000
N_CORES = 8
ROWS_PER_CORE = N // N_CORES          # 1024
N_CALLS = 8                           # prob gather calls per core
R = 128                               # idxs per gather call (1 row/partition)
ELEM = 128                            # f32 per gathered chunk (512 B)
CPR = C // ELEM                       # 250; max idx 127*250+249 = 31999 < 2^15
K_DVE = 6                             # fused selects on DVE; calls 6,7 split

_cached = None


def _build_bass():
    import concourse.bacc as bacc
    import concourse.mybir as mybir
    from contextlib import ExitStack

    f32 = mybir.dt.float32
    i16 = mybir.dt.int16
    A = mybir.AluOpType

    nc = bacc.Bacc(num_swdge_queues=4)
    prob_d = nc.declare_dram_parameter("prob", [ROWS_PER_CORE, C], f32, isOutput=False)
    # comb: cols 0:64 gather-idx table, cols 64:80 offsets (f32 bitcast), pad
    comb_d = nc.declare_dram_parameter("comb", [128, 128], i16, isOutput=False)
    out_d = nc.declare_dram_parameter("out", [128, 64], f32, isOutput=True)

    with ExitStack() as stack:
        e = stack.enter_context
        pf = e(nc.sbuf_tensor([128, 1], f32))
        gk = [e(nc.sbuf_tensor(f"gk{k}", [128, 1], f32)) for k in range(7)]
        fl = e(nc.sbuf_tensor([128, 1], f32))
        mm = e(nc.sbuf_tensor([128, 1], f32))
        j16 = e(nc.sbuf_tensor([128, 8], f32))
        idf = e(nc.sbuf_tensor([128, 8], f32))
        idn16 = e(nc.sbuf_tensor([128, 8], i16))
        fio = e(nc.sbuf_tensor([128, ELEM], f32))
        comb_sb = e(nc.sbuf_tensor([128, 1, 128], i16))
        gath_sb = e(nc.sbuf_tensor([128, N_CALLS, ELEM], f32))
        junk_v = e(nc.sbuf_tensor([128, ELEM], f32))
        mask_sb = [e(nc.sbuf_tensor(f"mask{n}", [128, ELEM], f32)) for n in range(2)]
        prod_sb = [e(nc.sbuf_tensor(f"prod{n}", [128, ELEM], f32)) for n in range(2)]
        scat_sb = e(nc.sbuf_tensor([128, 1, 64], f32))
        ps = e(nc.semaphore("ps"))
        ld0 = e(nc.semaphore("ld0"))
        gs = [e(nc.semaphore(f"gs{i}")) for i in range(N_CALLS)]
        vs = e(nc.semaphore("vs"))
        pm = e(nc.semaphore("pm"))
        fin = e(nc.semaphore("fin"))
        block = e(nc.Block(no_gpsimd_drain=True))

        offs_ap = comb_sb[:, 0, 64:80].bitcast(f32)  # [128, 8]
        FIO_READY = 20   # ps count after the fio iota (idn chain is 19 ops)
        MEMSET_DONE = 21

        @block.gpsimd
        def _(g):
            # identity idx table idn[p, j] = p%16 + 16j (identical in every
            # 16-partition block, as each gpsimd core pair's replica must be):
            # f32 is_ge ladder for p%16, converting copy to int16
            c = [0]

            def step(inst):
                c[0] += 1
                inst.then_inc(ps, 1)
                return c[0]

            step(g.iota(pf[:], pattern=[[0, 1]], base=0, channel_multiplier=1,
                        allow_small_or_imprecise_dtypes=True))
            step(g.iota(j16[:], pattern=[[16, 8]], base=0, channel_multiplier=0,
                        allow_small_or_imprecise_dtypes=True))
            g.wait_ge(ps, c[0])
            for k in range(7):
                step(g.tensor_scalar(gk[k][:], pf[:], 16.0 * (k + 1), None,
                                     op0=A.is_ge))
            g.wait_ge(ps, c[0])
            step(g.tensor_tensor(fl[:], gk[0][:], gk[1][:], op=A.add))
            for k in range(2, 7):
                g.wait_ge(ps, c[0])
                step(g.tensor_tensor(fl[:], fl[:], gk[k][:], op=A.add))
            g.wait_ge(ps, c[0])
            step(g.tensor_scalar(fl[:], fl[:], 16.0, None, op0=A.mult))
            g.wait_ge(ps, c[0])
            step(g.tensor_tensor(mm[:], pf[:], fl[:], op=A.subtract))
            g.wait_ge(ps, c[0])
            step(g.tensor_scalar(idf[:], j16[:], mm[:], None, op0=A.add))
            g.wait_ge(ps, c[0])
            step(g.tensor_copy(idn16[:], idf[:]))
            g.wait_ge(ps, c[0])
            # iota for the selects (0..127 along the free dim)
            step(g.iota(fio[:], pattern=[[1, ELEM]], base=0, channel_multiplier=0,
                        allow_small_or_imprecise_dtypes=True))
            assert c[0] == FIO_READY, c[0]
            g.dma_gather(
                comb_sb[:], comb_d[:], idn16[:],
                num_idxs=128, num_idxs_reg=128, elem_size=128, queue_num=0,
            ).then_inc(ld0, 16)
            g.wait_ge(ld0, 16)
            for i in range(N_CALLS):
                src = prob_d[R * i : R * (i + 1), :].rearrange(
                    "r (c e) -> (r c) e", e=ELEM
                )
                g.dma_gather(
                    gath_sb[:, i : i + 1], src,
                    comb_sb[:, 0, 8 * i : 8 * (i + 1)],
                    num_idxs=R, num_idxs_reg=R, elem_size=ELEM, queue_num=i % 4,
                ).then_inc(gs[i], 16)
            # scatter payload cols 8:64 must land as zeros; disjoint from the
            # pick columns, so this can run after the gathers
            step(g.memset(scat_sb[:, 0, N_CALLS:64], 0.0))
            # mask/product passes for calls 7 and 6 (the fused select form is
            # only legal on the vector engine; gpsimd feeds it instead)
            assert c[0] == MEMSET_DONE, c[0]
            g.wait_ge(ps, FIO_READY)
            for n, i in enumerate((7, 6)):
                g.wait_ge(gs[i], 16)
                step(g.tensor_scalar(mask_sb[n][:], fio[:],
                                     offs_ap[:, i : i + 1], None, op0=A.is_equal))
                g.wait_ge(ps, c[0])
                g.tensor_tensor(prod_sb[n][:], mask_sb[n][:],
                                gath_sb[:, i], op=A.mult).then_inc(pm, 1)
            g.wait_ge(ps, MEMSET_DONE)
            g.wait_ge(vs, N_CALLS)
            # output buffers are pre-zeroed by the runner on both the native
            # and PJRT paths, so the scatter-add lands the picked values as-is
            g.dma_scatter_add(
                out_d[:], scat_sb[:], idn16[:],
                num_idxs=128, num_idxs_reg=128, elem_size=64, queue_num=3,
            ).then_inc(fin, 16)
            g.wait_ge(fin, 16)

        @block.vector
        def _(v):
            v.wait_ge(ld0, 16)
            v.wait_ge(ps, FIO_READY)
            for i in range(K_DVE):
                v.wait_ge(gs[i], 16)
                if i > 0:
                    v.wait_ge(vs, i)
                v.scalar_tensor_tensor(
                    junk_v[:],
                    fio[:],
                    offs_ap[:, i : i + 1],
                    gath_sb[:, i],
                    op0=A.is_equal,
                    op1=A.mult,
                    accum_out=scat_sb[:, 0, i : i + 1],
                ).then_inc(vs, 1)
            for n, i in enumerate((7, 6)):
                v.wait_ge(pm, n + 1)
                v.wait_ge(vs, K_DVE + n)
                v.tensor_scalar(
                    junk_v[:], prod_sb[n][:], 1.0, 0.0, op0=A.mult, op1=A.add,
                    accum_out=scat_sb[:, 0, i : i + 1],
                ).then_inc(vs, 1)

    nc.compile()
    return nc


def _shard_host_inputs(prob, target, reward):
    """Per-core in_maps: prob shard + combined gather-idx/offset table."""
    t_all = np.asarray(target).astype(np.int64)
    prob = np.ascontiguousarray(np.asarray(prob, dtype=np.float32))
    loc = np.arange(R)
    in_maps = []
    for core in range(N_CORES):
        base = core * ROWS_PER_CORE
        t = t_all[base : base + ROWS_PER_CORE]
        chunk = (t // ELEM).astype(np.int64)
        off = (t % ELEM).astype(np.float32)
        gidx16 = np.zeros((16, 64), np.int16)
        offs = np.zeros((128, N_CALLS), np.float32)
        for i in range(N_CALLS):
            rb = R * i
            idxv = loc * CPR + chunk[rb + loc]
            gidx16[loc % 16, 8 * i + loc // 16] = idxv.astype(np.int16)
            offs[:, i] = off[rb : rb + 128]
        comb = np.zeros((128, 128), np.int16)
        comb[:, 0:64] = np.tile(gidx16, (8, 1))
        comb[:, 64:80] = offs.view(np.int16)
        in_maps.append(
            {
                "prob": prob[base : base + ROWS_PER_CORE],
                "comb": comb,
            }
        )
    return in_maps


def kernel(prob, target, reward):
    global _cached
    from concourse.bass_utils import run_bass_kernel_spmd

    if _cached is None:
        _cached = _build_bass()
    nc = _cached
    in_maps = _shard_host_inputs(prob, target, reward)
    res = run_bass_kernel_spmd(nc, in_maps, list(range(N_CORES)))
    r_all = np.asarray(reward, dtype=np.float64)
    total = np.float64(0.0)
    for core, core_out in enumerate(res.results):
        picked = np.asarray(core_out["out"], dtype=np.float64)[:, :N_CALLS]
        rew = r_all[core * ROWS_PER_CORE : (core + 1) * ROWS_PER_CORE]
        # picked[p, i] belongs to row 128*i + p of this core's shard
        total += (picked * rew.reshape(N_CALLS, 128).T).sum()
    return np.float32(-total)


# revision 36
# speedup vs baseline: 5.1695x; 1.0235x over previous
"""GANLoss kernel for Trainium2: out = -sum_i prob[i, target[i]] * reward[i].

Shapes: prob (8192, 32000) f32, target (8192,) int64, reward (8192,) f32.
Sharding: rows split across 8 NeuronCores (1024 rows/core).

Strategy: the loss touches one element per row. Per core, 8 dma_gather calls
of 128 idxs x 128 f32 (512 B) fetch the chunk holding each row's target
element; each call lands one row per partition, so one fused
scalar_tensor_tensor ((iota == offs) * chunk, accum_out=row-sum) on the
vector engine extracts the picked element for 128 rows in one pass. The
small input table (gather indices + offsets) is itself loaded with
dma_gather via an identity index table built on gpsimd in f32 (iota +
is_ge ladder for p%16, then a converting copy to int16 — the only integer
path the engines legally support), and the result leaves through
dma_scatter_add onto the runner's pre-zeroed output buffer — the kernel
issues no plain DMA at all, dodging the HWDGE descriptor path and its
end-of-kernel drain penalty. gpsimd runs the gathers, the last two selects'
mask/product passes, and the scatter; the vector engine runs six fused
selects plus two fast accumulate passes over gpsimd's products. The device
returns the 1024 picked probs per core; the host applies the reward weights
and the final negated sum.
"""

import numpy as np

N, C = 8192, 32000
N_CORES = 8
ROWS_PER_CORE = N // N_CORES          # 1024
N_CALLS = 8                           # prob gather calls per core
R = 128                               # idxs per gather call (1 row/partition)
ELEM = 128                            # f32 per gathered chunk (512 B)
CPR = C // ELEM                       # 250; max idx 127*250+249 = 31999 < 2^15
K_DVE = 6                             # fused selects on DVE; calls 6,7 split

_cached = None


def _build_bass():
    import concourse.bacc as bacc
    import concourse.mybir as mybir
    from contextlib import ExitStack

    f32 = mybir.dt.float32
    i16 = mybir.dt.int16
    A = mybir.AluOpType

    nc = bacc.Bacc(num_swdge_queues=4)
    prob_d = nc.declare_dram_parameter("prob", [ROWS_PER_CORE, C], f32, isOutput=False)
    # comb: cols 0:64 gather-idx table, cols 64:80 offsets (f32 bitcast), pad
    comb_d = nc.declare_dram_parameter("comb", [128, 128], i16, isOutput=False)
    out_d = nc.declare_dram_parameter("out", [128, 64], f32, isOutput=True)

    with ExitStack() as stack:
        e = stack.enter_context
        pf = e(nc.sbuf_tensor([128, 1], f32))
        gk = [e(nc.sbuf_tensor(f"gk{k}", [128, 1], f32)) for k in range(7)]
        fl = e(nc.sbuf_tensor([128, 1], f32))
        mm = e(nc.sbuf_tensor([128, 1], f32))
        j16 = e(nc.sbuf_tensor([128, 8], f32))
        idf = e(nc.sbuf_tensor([128, 8], f32))
        idn16 = e(nc.sbuf_tensor([128, 8], i16))
        fio = e(nc.sbuf_tensor([128, ELEM], f32))
        comb_sb = e(nc.sbuf_tensor([128, 1, 128], i16))
        gath_sb = e(nc.sbuf_tensor([128, N_CALLS, ELEM], f32))
        junk_v = e(nc.sbuf_tensor([128, ELEM], f32))
        mask_sb = [e(nc.sbuf_tensor(f"mask{n}", [128, ELEM], f32)) for n in range(2)]
        prod_sb = [e(nc.sbuf_tensor(f"prod{n}", [128, ELEM], f32)) for n in range(2)]
        scat_sb = e(nc.sbuf_tensor([128, 1, 64], f32))
        ps = e(nc.semaphore("ps"))
        ld0 = e(nc.semaphore("ld0"))
        gs = [e(nc.semaphore(f"gs{i}")) for i in range(N_CALLS)]
        vs = e(nc.semaphore("vs"))
        pm = e(nc.semaphore("pm"))
        fin = e(nc.semaphore("fin"))
        prep = e(nc.semaphore("prep"))
        block = e(nc.Block(no_gpsimd_drain=True))

        offs_ap = comb_sb[:, 0, 64:80].bitcast(f32)  # [128, 8]
        FIO_READY = 20   # ps count after the fio iota (idn chain is 19 ops)
        MEMSET_DONE = 21

        @block.gpsimd
        def _(g):
            # identity idx table idn[p, j] = p%16 + 16j (identical in every
            # 16-partition block, as each gpsimd core pair's replica must be):
            # f32 is_ge ladder for p%16, converting copy to int16
            c = [0]

            def step(inst):
                c[0] += 1
                inst.then_inc(ps, 1)
                return c[0]

            step(g.iota(pf[:], pattern=[[0, 1]], base=0, channel_multiplier=1,
                        allow_small_or_imprecise_dtypes=True))
            step(g.iota(j16[:], pattern=[[16, 8]], base=0, channel_multiplier=0,
                        allow_small_or_imprecise_dtypes=True))
            g.wait_ge(ps, c[0])
            for k in range(7):
                step(g.tensor_scalar(gk[k][:], pf[:], 16.0 * (k + 1), None,
                                     op0=A.is_ge))
            g.wait_ge(ps, c[0])
            step(g.tensor_tensor(fl[:], gk[0][:], gk[1][:], op=A.add))
            for k in range(2, 7):
                g.wait_ge(ps, c[0])
                step(g.tensor_tensor(fl[:], fl[:], gk[k][:], op=A.add))
            g.wait_ge(ps, c[0])
            step(g.tensor_scalar(fl[:], fl[:], 16.0, None, op0=A.mult))
            g.wait_ge(ps, c[0])
            step(g.tensor_tensor(mm[:], pf[:], fl[:], op=A.subtract))
            g.wait_ge(ps, c[0])
            step(g.tensor_scalar(idf[:], j16[:], mm[:], None, op0=A.add))
            g.wait_ge(ps, c[0])
            step(g.tensor_copy(idn16[:], idf[:]))
            g.wait_ge(ps, c[0])
            # iota for the selects (0..127 along the free dim)
            step(g.iota(fio[:], pattern=[[1, ELEM]], base=0, channel_multiplier=0,
                        allow_small_or_imprecise_dtypes=True))
            assert c[0] == FIO_READY, c[0]
            g.dma_gather(
                comb_sb[:], comb_d[:], idn16[:],
                num_idxs=128, num_idxs_reg=128, elem_size=128, queue_num=0,
            ).then_inc(ld0, 16)
            g.wait_ge(ld0, 16)
            for i in range(N_CALLS):
                src = prob_d[R * i : R * (i + 1), :].rearrange(
                    "r (c e) -> (r c) e", e=ELEM
                )
                g.dma_gather(
                    gath_sb[:, i : i + 1], src,
                    comb_sb[:, 0, 8 * i : 8 * (i + 1)],
                    num_idxs=R, num_idxs_reg=R, elem_size=ELEM, queue_num=i % 4,
                ).then_inc(gs[i], 16)
            # scatter payload cols 8:64 must land as zeros; disjoint from the
            # pick columns, so this can run after the gathers
            step(g.memset(scat_sb[:, 0, N_CALLS:64], 0.0))
            # mask/product passes for calls 7 and 6 (the fused select form is
            # only legal on the vector engine; gpsimd feeds it instead)
            assert c[0] == MEMSET_DONE, c[0]
            g.wait_ge(ps, FIO_READY)
            for n, i in enumerate((7, 6)):
                g.wait_ge(gs[i], 16)
                step(g.tensor_scalar(mask_sb[n][:], fio[:],
                                     offs_ap[:, i : i + 1], None, op0=A.is_equal))
                g.wait_ge(ps, c[0])
                g.tensor_tensor(prod_sb[n][:], mask_sb[n][:],
                                gath_sb[:, i], op=A.mult).then_inc(pm, 1)
            # pre-generate the scatter descriptors; the DMA reads the
            # payload only when triggered after the selects complete
            g.dma_scatter_add(
                out_d[:], scat_sb[:], idn16[:],
                num_idxs=128, num_idxs_reg=128, elem_size=64, queue_num=3,
                prepare_only=True, sem=fin,
            ).then_inc(prep, 1)
            g.wait_ge(prep, 1)
            g.wait_ge(ps, MEMSET_DONE)
            g.wait_ge(vs, N_CALLS)
            # output buffers are pre-zeroed by the runner on both the native
            # and PJRT paths, so the scatter-add lands the picked values as-is
            g.trigger_dma(count=1, queue_num=3)
            g.wait_ge(fin, 16)

        @block.vector
        def _(v):
            v.wait_ge(ld0, 16)
            v.wait_ge(ps, FIO_READY)
            for i in range(K_DVE):
                v.wait_ge(gs[i], 16)
                if i > 0:
                    v.wait_ge(vs, i)
                v.scalar_tensor_tensor(
                    junk_v[:],
                    fio[:],
                    offs_ap[:, i : i + 1],
                    gath_sb[:, i],
                    op0=A.is_equal,
                    op1=A.mult,
                    accum_out=scat_sb[:, 0, i : i + 1],
                ).then_inc(vs, 1)
            for n, i in enumerate((7, 6)):
                v.wait_ge(pm, n + 1)
                v.wait_ge(vs, K_DVE + n)
                v.tensor_scalar(
                    junk_v[:], prod_sb[n][:], 1.0, 0.0, op0=A.mult, op1=A.add,
                    accum_out=scat_sb[:, 0, i : i + 1],
                ).then_inc(vs, 1)

    nc.compile()
    return nc


def _shard_host_inputs(prob, target, reward):
    """Per-core in_maps: prob shard + combined gather-idx/offset table."""
    t_all = np.asarray(target).astype(np.int64)
    prob = np.ascontiguousarray(np.asarray(prob, dtype=np.float32))
    loc = np.arange(R)
    in_maps = []
    for core in range(N_CORES):
        base = core * ROWS_PER_CORE
        t = t_all[base : base + ROWS_PER_CORE]
        chunk = (t // ELEM).astype(np.int64)
        off = (t % ELEM).astype(np.float32)
        gidx16 = np.zeros((16, 64), np.int16)
        offs = np.zeros((128, N_CALLS), np.float32)
        for i in range(N_CALLS):
            rb = R * i
            idxv = loc * CPR + chunk[rb + loc]
            gidx16[loc % 16, 8 * i + loc // 16] = idxv.astype(np.int16)
            offs[:, i] = off[rb : rb + 128]
        comb = np.zeros((128, 128), np.int16)
        comb[:, 0:64] = np.tile(gidx16, (8, 1))
        comb[:, 64:80] = offs.view(np.int16)
        in_maps.append(
            {
                "prob": prob[base : base + ROWS_PER_CORE],
                "comb": comb,
            }
        )
    return in_maps


def kernel(prob, target, reward):
    global _cached
    from concourse.bass_utils import run_bass_kernel_spmd

    if _cached is None:
        _cached = _build_bass()
    nc = _cached
    in_maps = _shard_host_inputs(prob, target, reward)
    res = run_bass_kernel_spmd(nc, in_maps, list(range(N_CORES)))
    r_all = np.asarray(reward, dtype=np.float64)
    total = np.float64(0.0)
    for core, core_out in enumerate(res.results):
        picked = np.asarray(core_out["out"], dtype=np.float64)[:, :N_CALLS]
        rew = r_all[core * ROWS_PER_CORE : (core + 1) * ROWS_PER_CORE]
        # picked[p, i] belongs to row 128*i + p of this core's shard
        total += (picked * rew.reshape(N_CALLS, 128).T).sum()
    return np.float32(-total)


# revision 38
# speedup vs baseline: 5.3105x; 1.0273x over previous
"""GANLoss kernel for Trainium2: out = -sum_i prob[i, target[i]] * reward[i].

Shapes: prob (8192, 32000) f32, target (8192,) int64, reward (8192,) f32.
Sharding: rows split across 8 NeuronCores (1024 rows/core).

Strategy: the loss touches one element per row. Per core, 8 dma_gather calls
of 128 idxs x 128 f32 (512 B) fetch the chunk holding each row's target
element; each call lands one row per partition, so one fused
scalar_tensor_tensor ((iota == offs) * chunk, accum_out=row-sum) on the
vector engine extracts the picked element for 128 rows in one pass. The
small input table (gather indices + offsets) is itself loaded with
dma_gather via an identity index table built on gpsimd in f32 (iota +
is_ge ladder for p%16, then a converting copy to int16 — the only integer
path the engines legally support), and the result leaves through
dma_scatter_add onto the runner's pre-zeroed output buffer — the kernel
issues no plain DMA at all, dodging the HWDGE descriptor path and its
end-of-kernel drain penalty. gpsimd runs the gathers, the last two selects'
mask/product passes, and the scatter; the vector engine runs six fused
selects plus two fast accumulate passes over gpsimd's products. The device
returns the 1024 picked probs per core; the host applies the reward weights
and the final negated sum.
"""

import numpy as np

N, C = 8192, 32000
N_CORES = 8
ROWS_PER_CORE = N // N_CORES          # 1024
N_CALLS = 8                           # prob gather calls per core
R = 128                               # idxs per gather call (1 row/partition)
ELEM = 128                            # f32 per gathered chunk (512 B)
CPR = C // ELEM                       # 250; max idx 127*250+249 = 31999 < 2^15
K_DVE = 6                             # fused selects on DVE; calls 6,7 split

_cached = None


def _build_bass():
    import concourse.bacc as bacc
    import concourse.mybir as mybir
    from contextlib import ExitStack

    f32 = mybir.dt.float32
    i16 = mybir.dt.int16
    A = mybir.AluOpType

    nc = bacc.Bacc(num_swdge_queues=4)
    prob_d = nc.declare_dram_parameter("prob", [ROWS_PER_CORE, C], f32, isOutput=False)
    # comb: cols 0:64 gather-idx table, cols 64:80 offsets (f32 bitcast), pad
    comb_d = nc.declare_dram_parameter("comb", [128, 128], i16, isOutput=False)
    mr7_d = nc.declare_dram_parameter("mr7", [128, 128], f32, isOutput=False)
    mr6_d = nc.declare_dram_parameter("mr6", [128, 128], f32, isOutput=False)
    out_d = nc.declare_dram_parameter("out", [128, 64], f32, isOutput=True)

    with ExitStack() as stack:
        e = stack.enter_context
        pf = e(nc.sbuf_tensor([128, 1], f32))
        gk = [e(nc.sbuf_tensor(f"gk{k}", [128, 1], f32)) for k in range(7)]
        fl = e(nc.sbuf_tensor([128, 1], f32))
        mm = e(nc.sbuf_tensor([128, 1], f32))
        j16 = e(nc.sbuf_tensor([128, 8], f32))
        idf = e(nc.sbuf_tensor([128, 8], f32))
        idn16 = e(nc.sbuf_tensor([128, 8], i16))
        fio = e(nc.sbuf_tensor([128, ELEM], f32))
        comb_sb = e(nc.sbuf_tensor([128, 1, 128], i16))
        gath_sb = e(nc.sbuf_tensor([128, N_CALLS, ELEM], f32))
        junk_v = e(nc.sbuf_tensor([128, ELEM], f32))
        mr_sb = [e(nc.sbuf_tensor(f"mr{n}", [128, 1, ELEM], f32)) for n in range(2)]
        prod_sb = e(nc.sbuf_tensor([128, 2, ELEM], f32))
        junk2 = e(nc.sbuf_tensor([128, 2 * ELEM], f32))
        scat_sb = e(nc.sbuf_tensor([128, 1, 64], f32))
        ps = e(nc.semaphore("ps"))
        ld0 = e(nc.semaphore("ld0"))
        gs = [e(nc.semaphore(f"gs{i}")) for i in range(N_CALLS)]
        vs = e(nc.semaphore("vs"))
        pm = e(nc.semaphore("pm"))
        mls = [e(nc.semaphore(f"ml{n}")) for n in range(2)]
        fin = e(nc.semaphore("fin"))
        prep = e(nc.semaphore("prep"))
        block = e(nc.Block(no_gpsimd_drain=True))

        offs_ap = comb_sb[:, 0, 64:80].bitcast(f32)  # [128, 8]
        FIO_READY = 20   # ps count after the fio iota (idn chain is 19 ops)
        MEMSET_DONE = 21

        @block.gpsimd
        def _(g):
            # identity idx table idn[p, j] = p%16 + 16j (identical in every
            # 16-partition block, as each gpsimd core pair's replica must be):
            # f32 is_ge ladder for p%16, converting copy to int16
            c = [0]

            def step(inst):
                c[0] += 1
                inst.then_inc(ps, 1)
                return c[0]

            step(g.iota(pf[:], pattern=[[0, 1]], base=0, channel_multiplier=1,
                        allow_small_or_imprecise_dtypes=True))
            step(g.iota(j16[:], pattern=[[16, 8]], base=0, channel_multiplier=0,
                        allow_small_or_imprecise_dtypes=True))
            g.wait_ge(ps, c[0])
            for k in range(7):
                step(g.tensor_scalar(gk[k][:], pf[:], 16.0 * (k + 1), None,
                                     op0=A.is_ge))
            g.wait_ge(ps, c[0])
            step(g.tensor_tensor(fl[:], gk[0][:], gk[1][:], op=A.add))
            for k in range(2, 7):
                g.wait_ge(ps, c[0])
                step(g.tensor_tensor(fl[:], fl[:], gk[k][:], op=A.add))
            g.wait_ge(ps, c[0])
            step(g.tensor_scalar(fl[:], fl[:], 16.0, None, op0=A.mult))
            g.wait_ge(ps, c[0])
            step(g.tensor_tensor(mm[:], pf[:], fl[:], op=A.subtract))
            g.wait_ge(ps, c[0])
            step(g.tensor_scalar(idf[:], j16[:], mm[:], None, op0=A.add))
            g.wait_ge(ps, c[0])
            step(g.tensor_copy(idn16[:], idf[:]))
            g.wait_ge(ps, c[0])
            # iota for the selects (0..127 along the free dim)
            step(g.iota(fio[:], pattern=[[1, ELEM]], base=0, channel_multiplier=0,
                        allow_small_or_imprecise_dtypes=True))
            assert c[0] == FIO_READY, c[0]
            g.dma_gather(
                comb_sb[:], comb_d[:], idn16[:],
                num_idxs=128, num_idxs_reg=128, elem_size=128, queue_num=0,
            ).then_inc(ld0, 16)
            g.wait_ge(ld0, 16)
            mr_d = [mr7_d, mr6_d]

            def gcall(i):
                psrc = prob_d[R * i : R * (i + 1), :].rearrange(
                    "r (c e) -> (r c) e", e=ELEM
                )
                g.dma_gather(
                    gath_sb[:, i : i + 1], psrc,
                    comb_sb[:, 0, 8 * i : 8 * (i + 1)],
                    num_idxs=R, num_idxs_reg=R, elem_size=ELEM, queue_num=i % 4,
                ).then_inc(gs[i], 16)

            # host maskrew loads ride in the gather-slack the vector engine's
            # 194ns/select pace leaves over gpsimd's 107ns/gather
            gcall(0)
            gcall(1)
            g.dma_gather(
                mr_sb[0][:], mr7_d[:], idn16[:],
                num_idxs=128, num_idxs_reg=128, elem_size=ELEM, queue_num=1,
            ).then_inc(mls[0], 16)
            gcall(2)
            g.dma_gather(
                mr_sb[1][:], mr6_d[:], idn16[:],
                num_idxs=128, num_idxs_reg=128, elem_size=ELEM, queue_num=2,
            ).then_inc(mls[1], 16)
            for i in range(3, N_CALLS):
                gcall(i)
            # products for calls 7 and 6 (reward folded into the host masks,
            # so one merged accumulate on the vector engine covers both)
            for n, i in enumerate((7, 6)):
                g.wait_ge(gs[i], 16)
                g.wait_ge(mls[n], 16)
                g.tensor_tensor(prod_sb[:, n], mr_sb[n][:, 0],
                                gath_sb[:, i], op=A.mult).then_inc(pm, 1)
            # scatter payload cols 7:64 must land as zeros; disjoint from the
            # pick columns, so this can run after the gathers
            step(g.memset(scat_sb[:, 0, 7:64], 0.0))
            assert c[0] == MEMSET_DONE, c[0]
            # pre-generate the scatter descriptors; the DMA reads the
            # payload only when triggered after the selects complete
            g.dma_scatter_add(
                out_d[:], scat_sb[:], idn16[:],
                num_idxs=128, num_idxs_reg=128, elem_size=64, queue_num=3,
                prepare_only=True, sem=fin,
            ).then_inc(prep, 1)
            g.wait_ge(prep, 1)
            g.wait_ge(ps, MEMSET_DONE)
            g.wait_ge(vs, K_DVE + 1)
            # output buffers are pre-zeroed by the runner on both the native
            # and PJRT paths, so the scatter-add lands the picked values as-is
            g.trigger_dma(count=1, queue_num=3)
            g.wait_ge(fin, 16)

        @block.vector
        def _(v):
            v.wait_ge(ld0, 16)
            v.wait_ge(ps, FIO_READY)
            for i in range(K_DVE):
                v.wait_ge(gs[i], 16)
                if i > 0:
                    v.wait_ge(vs, i)
                v.scalar_tensor_tensor(
                    junk_v[:],
                    fio[:],
                    offs_ap[:, i : i + 1],
                    gath_sb[:, i],
                    op0=A.is_equal,
                    op1=A.mult,
                    accum_out=scat_sb[:, 0, i : i + 1],
                ).then_inc(vs, 1)
            v.wait_ge(pm, 2)
            v.wait_ge(vs, K_DVE)
            v.tensor_scalar(
                junk2[:], prod_sb[:].rearrange("p a b -> p (a b)"),
                1.0, 0.0, op0=A.mult, op1=A.add,
                accum_out=scat_sb[:, 0, 6:7],
            ).then_inc(vs, 1)

    nc.compile()
    return nc


def _shard_host_inputs(prob, target, reward):
    """Per-core in_maps: prob shard + combined gather-idx/offset table."""
    t_all = np.asarray(target).astype(np.int64)
    prob = np.ascontiguousarray(np.asarray(prob, dtype=np.float32))
    loc = np.arange(R)
    in_maps = []
    for core in range(N_CORES):
        base = core * ROWS_PER_CORE
        t = t_all[base : base + ROWS_PER_CORE]
        chunk = (t // ELEM).astype(np.int64)
        off = (t % ELEM).astype(np.float32)
        gidx16 = np.zeros((16, 64), np.int16)
        offs = np.zeros((128, N_CALLS), np.float32)
        for i in range(N_CALLS):
            rb = R * i
            idxv = loc * CPR + chunk[rb + loc]
            gidx16[loc % 16, 8 * i + loc // 16] = idxv.astype(np.int16)
            offs[:, i] = off[rb : rb + 128]
        comb = np.zeros((128, 128), np.int16)
        comb[:, 0:64] = np.tile(gidx16, (8, 1))
        comb[:, 64:80] = offs.view(np.int16)
        r_all = np.asarray(reward, dtype=np.float32)
        mrs = []
        for i in (7, 6):
            mr = np.zeros((128, ELEM), np.float32)
            rows = base + R * i + np.arange(128)
            mr[np.arange(128), (t_all[rows] % ELEM)] = r_all[rows]
            mrs.append(mr)
        in_maps.append(
            {
                "prob": prob[base : base + ROWS_PER_CORE],
                "comb": comb,
                "mr7": mrs[0],
                "mr6": mrs[1],
            }
        )
    return in_maps


def kernel(prob, target, reward):
    global _cached
    from concourse.bass_utils import run_bass_kernel_spmd

    if _cached is None:
        _cached = _build_bass()
    nc = _cached
    in_maps = _shard_host_inputs(prob, target, reward)
    res = run_bass_kernel_spmd(nc, in_maps, list(range(N_CORES)))
    r_all = np.asarray(reward, dtype=np.float64)
    total = np.float64(0.0)
    for core, core_out in enumerate(res.results):
        out = np.asarray(core_out["out"], dtype=np.float64)
        rew = r_all[core * ROWS_PER_CORE : (core + 1) * ROWS_PER_CORE]
        # cols 0:6 hold raw picks for rows 128*i + p; col 6 holds the merged
        # reward-weighted contribution of calls 6 and 7
        rmat = rew.reshape(N_CALLS, 128).T
        total += (out[:, :K_DVE] * rmat[:, :K_DVE]).sum() + out[:, K_DVE].sum()
    return np.float32(-total)
